# revision 3
# baseline (speedup 1.0000x reference)
"""EuclideanDeconf kernel for 8x TRN2 NeuronCores.

Computes out[b, c] = (2/D) * x @ W.T - ||x||^2/D - ||W||^2/D
for x [16384, 1024] f32, W [2048, 1024] f32 -> out [16384, 2048] f32.

Sharding: data-parallel over the batch dim. Each of the 8 cores gets 2048
rows of x (passed pre-transposed as xT [1024, 2048] f32) and the full W
(passed pre-transposed and bf16-cast as wT [1024, 2048] bf16). The host does
layout-only work (transpose / cast / shard / concat); all FLOPs (matmul,
row/col norms, combine) run on device.

Numerics: cross term in bf16 (its magnitude is ~0.003 of the ~1.0 output, so
bf16 rounding contributes ~1e-5 relative error); x2 computed on-device in
fp32 from the fp32 x (the dominant term, kept exact); w2 from bf16 W (w2 is
~0.002 so its rounding is ~1e-5 absolute).
"""

import numpy as np
import ml_dtypes

# Problem constants (hardcoded; kernel.py must be self-contained).
B, D, C = 16384, 1024, 2048
NCORES = 8
BSH = B // NCORES  # 2048 rows of x per core
P = 128            # partitions
KT = D // P        # 8 contraction tiles
BCH = 512          # b-chunk (columns of xT loaded per DMA)
NTB = BSH // P     # 16 b-tiles per core

_CACHE = {}


def _build_nc():
    import concourse.tile as tile
    import concourse.mybir as mybir
    from concourse import bacc

    f32 = mybir.dt.float32
    bf16 = mybir.dt.bfloat16
    PSUM = __import__("concourse.bass", fromlist=["MemorySpace"]).MemorySpace.PSUM
    Square = mybir.ActivationFunctionType.Square
    Identity = mybir.ActivationFunctionType.Identity
    X = mybir.AxisListType.X
    ADD = mybir.AluOpType.add

    nc = bacc.Bacc(
        "TRN2",
        target_bir_lowering=False,
        debug=False,
        enable_asserts=False,
    )
    xT = nc.dram_tensor("xT", [D, BSH], f32, kind="ExternalInput").ap()
    wT = nc.dram_tensor("wT", [D, C], bf16, kind="ExternalInput").ap()
    y = nc.dram_tensor("y", [BSH, C], f32, kind="ExternalOutput").ap()

    SC = 1.0 / 32.0  # sqrt(1/D): Square(in*SC) = in^2/D

    with tile.TileContext(nc) as tc:
        with (
            tc.tile_pool(name="consts", bufs=1) as cpool,
            tc.tile_pool(name="wpool", bufs=1) as wpool,
            tc.tile_pool(name="xpool", bufs=2) as xpool,
            tc.tile_pool(name="xsqpool", bufs=2) as xsqpool,
            tc.tile_pool(name="epool", bufs=3) as epool,
            tc.tile_pool(name="ypool", bufs=2) as ypool,
            tc.tile_pool(name="spool", bufs=4) as spool,
            tc.tile_pool(name="pmain", bufs=3, space=PSUM) as pmain,
            tc.tile_pool(name="psmall", bufs=2, space=PSUM) as psmall,
        ):
            negones_f = cpool.tile([P, 1], f32)
            nc.gpsimd.memset(negones_f[:], -1.0)
            negones_b = cpool.tile([P, 1], bf16)
            nc.gpsimd.memset(negones_b[:], -1.0)
            ones1_b = cpool.tile([1, P], bf16)
            nc.gpsimd.memset(ones1_b[:], 1.0)

            # --- Load W (bf16, transposed) resident in SBUF ---
            wbf = wpool.tile([P, KT, C], bf16)
            nc.sync.dma_start(wbf[:], wT.rearrange("(k p) c -> p k c", p=P))

            # --- w2[c] = ||W[c]||^2 / D, negated + replicated to [128, C] ---
            # PSUM comes from the same slots the main loop uses (tag "ps" /
            # "w2rp" in psmall), so no extra banks are reserved.
            w2rep = wpool.tile([P, C], f32)
            with tc.tile_pool(name="w2tmp", bufs=2) as w2tmp:
                w2ps0 = pmain.tile([1, 1024], f32, tag="ps")
                w2ps1 = pmain.tile([1, 1024], f32, tag="ps")
                w2pss = (w2ps0, w2ps0, w2ps1, w2ps1)
                for k in range(KT):
                    wsq = w2tmp.tile([P, C], bf16, tag="wsq")
                    nc.scalar.activation(wsq[:], wbf[:, k, :], Square, scale=SC)
                    for cj in range(C // 512):
                        # psum[0, c] -= sum_p wsq[p, c]
                        nc.tensor.matmul(
                            w2pss[cj][:, (cj % 2) * 512:(cj % 2) * 512 + 512],
                            negones_b[:],
                            wsq[:, cj * 512:(cj + 1) * 512],
                            start=(k == 0),
                            stop=(k == KT - 1),
                        )
                w2row = w2tmp.tile([1, C], bf16, tag="w2row")
                nc.vector.tensor_copy(w2row[:, 0:1024], w2ps0[:])
                nc.vector.tensor_copy(w2row[:, 1024:2048], w2ps1[:])
                for cj in range(C // 512):
                    w2rp = psmall.tile([P, 512], f32, tag="w2rp", bufs=1)
                    nc.tensor.matmul(
                        w2rp[:],
                        ones1_b[:],
                        w2row[:, cj * 512:(cj + 1) * 512],
                        start=True,
                        stop=True,
                    )
                    nc.vector.tensor_copy(w2rep[:, cj * 512:(cj + 1) * 512], w2rp[:])

            # --- Main loop over batch chunks ---
            for ch in range(BSH // BCH):
                xf = xpool.tile([P, KT, BCH], f32, tag="xf")
                nc.sync.dma_start(
                    xf[:],
                    xT[:, ch * BCH:(ch + 1) * BCH].rearrange("(k p) b -> p k b", p=P),
                )
                xbf = xpool.tile([P, KT, BCH], bf16, tag="xbf")
                nc.vector.tensor_copy(xbf[:], xf[:])
                # x^2 / D, then partial-reduce over the KT sub-rows
                xsq = xsqpool.tile([P, KT, BCH], f32, tag="xsq")
                nc.scalar.activation(xsq[:], xf[:], Square, scale=SC)
                xsqk = spool.tile([P, BCH], f32, tag="xsqk")
                nc.vector.tensor_reduce(
                    xsqk[:], xsq[:].rearrange("p k b -> p b k"), axis=X, op=ADD
                )

                for jj in range(BCH // P):
                    j = ch * (BCH // P) + jj
                    # x2col[b] = -sum_p xsqk[p, b]  (column vector via PE dot)
                    x2ps = psmall.tile([P, 1], f32, tag="x2ps", bufs=1)
                    nc.tensor.matmul(
                        x2ps[:],
                        xsqk[:, jj * P:(jj + 1) * P],
                        negones_f[:],
                        start=True,
                        stop=True,
                    )
                    x2col = spool.tile([P, 1], f32, tag="x2col")
                    nc.vector.tensor_copy(x2col[:], x2ps[:])

                    y_t = ypool.tile([P, C], f32, tag="y_t")
                    ps0 = pmain.tile([P, 1024], f32, tag="ps")
                    ps1 = pmain.tile([P, 1024], f32, tag="ps")
                    pss = (ps0, ps0, ps1, ps1)
                    for k in range(KT):
                        lhsT = xbf[:, k, jj * P:(jj + 1) * P]
                        for cj in range(4):
                            nc.tensor.matmul(
                                pss[cj][:, (cj % 2) * 512:(cj % 2) * 512 + 512],
                                lhsT,
                                wbf[:, k, cj * 512:(cj + 1) * 512],
                                start=(k == 0),
                                stop=(k == KT - 1),
                            )
                    for h, psh in enumerate((ps0, ps1)):
                        t = epool.tile([P, 1024], f32, tag="t")
                        # t = (2/D)*psum - x2  (scale + per-partition bias)
                        nc.scalar.activation(
                            t[:], psh[:], Identity, bias=x2col[:], scale=2.0 / D
                        )
                        # y = t - w2  (w2rep already negated)
                        nc.vector.tensor_add(
                            y_t[:, h * 1024:(h + 1) * 1024],
                            t[:],
                            w2rep[:, h * 1024:(h + 1) * 1024],
                        )
                    nc.sync.dma_start(y[j * P:(j + 1) * P, :], y_t[:])

    nc.compile()
    return nc


def _get_nc():
    if "nc" not in _CACHE:
        _CACHE["nc"] = _build_nc()
    return _CACHE["nc"]


def _prep_inputs(x, W):
    x = np.ascontiguousarray(x, dtype=np.float32)
    W = np.ascontiguousarray(W, dtype=np.float32)
    wT = np.ascontiguousarray(W.T).astype(ml_dtypes.bfloat16)
    in_maps = []
    for i in range(NCORES):
        xT_i = np.ascontiguousarray(x[i * BSH:(i + 1) * BSH, :].T)
        in_maps.append({"xT": xT_i, "wT": wT})
    return in_maps


def run(x, W, trace=False, **trace_kwargs):
    """Run on the 8 cores; returns (out [B, C] f32, BassKernelResults)."""
    from concourse import bass_utils

    nc = _get_nc()
    in_maps = _prep_inputs(x, W)
    res = bass_utils.run_bass_kernel_spmd(
        nc, in_maps, core_ids=list(range(NCORES)), trace=trace, **trace_kwargs
    )
    out = np.concatenate([r["y"] for r in res.results], axis=0)
    return out, res


def kernel(x, W, task_id=None, **_unused):
    out, _ = run(np.asarray(x), np.asarray(W), trace=False)
    return out
